# revision 11
# baseline (speedup 1.0000x reference)
"""CIN (Compressed Interaction Network) Trainium2 kernel.

Baseline architecture (pair-chunk L2, host-streamed replicated tiles)
with DMA-efficiency and scheduling improvements:
  * 2-granule pre-interleaved host layouts: each stream DMA trigger
    moves 512KB ([128, 2*CB] contiguous per partition row) — halves
    the ~610ns-per-trigger issue cost on the DGE queues.
  * w1 staged via the GpSimd SWDGE queue at t=0 (needed by block 0's
    L2); w2/ident deferred behind block 0's stream DMAs (first needed
    by the l3blk=0 pass during block 1).
  * per-block single-trigger xdiag slices.
See the baseline kernel docstring for the full algorithm description.
"""

import sys

import numpy as np

try:
    import concourse.bass as bass  # noqa: F401
except ImportError:  # grading env fallback
    sys.path.insert(0, "/opt/trn_rl_repo")

import ml_dtypes
import concourse.bacc as bacc
import concourse.bass as bass
import concourse.mybir as mybir
import concourse.tile as tile
from concourse.bass_utils import run_bass_kernel_spmd

BF16 = mybir.dt.bfloat16
F32 = mybir.dt.float32

B, F0, D = 2048, 64, 16
NCORES = 8
BL = B // NCORES          # 256 batch rows per core
C = BL * D                # 4096 columns (b, d)
FN = 128                  # layer width (all three CIN layers)
CT = 512                  # matmul N tile (one PSUM bank of fp32)
CB = 1024                 # column block
NBLK = C // CB            # 4
NCT = CB // CT            # 2 column tiles per block
NPAIR = F0 // 2           # 32 pair tiles / L2 pair groups
NG = BL // 8              # 32 groups of 8 batch rows (layer-3 path)
NGB = CB // 128           # 8 layer-3 groups per block
NBH = NBLK // 2           # 2 blocks per layer-3 half
SYM_PAIRS = F0 * (F0 + 1) // 2          # 2080 unordered (i,j) pairs
L1_CHUNKS = (SYM_PAIRS + 127) // 128    # 17 (last chunk zero-padded)
L2_CHUNKS = F0                 # 64 (pair x j-half)

NSD = NPAIR // 2               # stream DMAs per block (2 pairs each): 16
NP1D = (L1_CHUNKS + 1) // 2    # xp1 DMAs per block (2 chunks each): 9

_CACHE = {}


def _build_program():
    nc = bacc.Bacc(None, target_bir_lowering=False)

    xp1_d = nc.dram_tensor("xp1", [NBLK, NP1D, 128, 2 * CB], BF16, kind="ExternalInput")
    xtp_d = nc.dram_tensor("xtp", [NBLK, NSD, 128, 2 * CB], BF16, kind="ExternalInput")
    xdiag_d = nc.dram_tensor("xdiag", [NBLK, 128, NGB * 512], BF16, kind="ExternalInput")
    w0_d = nc.dram_tensor("w0c", [128, L1_CHUNKS * FN], BF16, kind="ExternalInput")
    w1_d = nc.dram_tensor("w1c", [128, L2_CHUNKS * FN], BF16, kind="ExternalInput")
    w2_d = nc.dram_tensor("w2c", [128, F0 * FN], BF16, kind="ExternalInput")
    ident_d = nc.dram_tensor("ident", [128, 128], BF16, kind="ExternalInput")
    out_d = nc.dram_tensor("out_nb", [3, 128, BL], F32, kind="ExternalOutput")

    with tile.TileContext(nc) as tc:
        with (
            tc.tile_pool(name="const", bufs=1) as const,
            tc.tile_pool(name="hbuf", bufs=1) as hbuf,
            tc.tile_pool(name="outs", bufs=1) as outs,
            tc.tile_pool(name="p1s", bufs=7) as p1s,
            tc.tile_pool(name="pairs", bufs=7) as pairs,
            tc.tile_pool(name="h2x", bufs=2) as h2xp,
            tc.tile_pool(name="pkr", bufs=6) as pkr,
            tc.tile_pool(name="zp", bufs=5, space="PSUM") as zp,
            tc.tile_pool(name="l3sb", bufs=1) as l3sb,
            tc.tile_pool(name="l3ps", bufs=2, space="PSUM") as l3ps,
            tc.tile_pool(name="o3p", bufs=1, space="PSUM") as o3p,
            tc.tile_pool(name="hts", bufs=6) as hts,
            tc.tile_pool(name="xdg", bufs=2) as xdg,
        ):
            w0_sb = const.tile([128, L1_CHUNKS * FN], BF16)
            nc.scalar.dma_start(w0_sb[:], w0_d[:])
            w1_sb = const.tile([128, L2_CHUNKS * FN], BF16)
            nc.gpsimd.dma_start(w1_sb[:], w1_d[:])
            w2_sb = const.tile([128, F0 * FN], BF16)
            ident_sb = const.tile([128, 128], BF16)

            h2_sb = hbuf.tile([128, C], BF16, tag="h2")
            out_sb = outs.tile([128, 3 * BL], F32)

            # dense junk-matmul burst at kernel start: pulls the PE HAM
            # clock gate to 8/8 before the real accumulation chains begin.
            warm_sb = const.tile([128, 512], BF16)
            nc.vector.memset(warm_sb[:], 0.0)
            warm_ps = zp.tile([128, CT], F32, tag="z", name="warm_ps")
            for w in range(20):
                nc.tensor.matmul(
                    warm_ps[:],
                    warm_sb[:, 0:128],
                    warm_sb[:],
                    start=(w == 0),
                    stop=(w == 19),
                )

            def alloc_z1(blk):
                return [
                    zp.tile([128, CT], F32, tag="z", name=f"z1_{blk}_{ct}")
                    for ct in range(NCT)
                ]

            p1_tiles = {}

            def emit_l1_dma(blk, u):
                last = u == NP1D - 1 and L1_CHUNKS % 2 == 1
                w = CB if last else 2 * CB
                p1 = p1s.tile([128, w], BF16, tag="p1", name=f"p1_{blk}_{u}")
                q = nc.sync if u % 2 == 0 else nc.scalar
                q.dma_start(p1[:], xp1_d[blk, u][:, 0:w])
                p1_tiles[(blk, u)] = p1

            def emit_l1_mm(blk, z1, t):
                p1 = p1_tiles[(blk, t // 2)]
                sl = p1[:, (t % 2) * CB : (t % 2) * CB + CB]
                for ct in range(NCT):
                    nc.tensor.matmul(
                        z1[ct][:],
                        w0_sb[:, t * FN : (t + 1) * FN],
                        sl[:, ct * CT : (ct + 1) * CT],
                        start=(t == 0),
                        stop=(t == L1_CHUNKS - 1),
                    )

            g2t_tiles = {}
            z1_cur = alloc_z1(0)
            for u in range(NP1D):
                emit_l1_dma(0, u)
            for t in range(L1_CHUNKS):
                emit_l1_mm(0, z1_cur, t)

            xd_tiles = {}
            for blk in range(NBLK):
                c0 = blk * CB
                half_idx = blk // NBH         # layer-3 half (0 or 1)
                if blk % NBH == 0:
                    g2t_tiles[half_idx] = l3sb.tile(
                        [128, NBH * NGB * 512],
                        BF16,
                        tag="g2t",
                        name=f"g2t_{half_idx}",
                    )
                z1 = z1_cur

                # z1 copy-out writes straight into the H2x duplication tile
                # (columns 0:CB hold the j<64 half, CB:2CB the j>=64 half);
                # one SBUF->SBUF DMA then fills partitions 64:128.
                h2x = h2xp.tile([128, 2 * CB], BF16, tag="h2x", name=f"h2x_{blk}")
                for ct in range(NCT):
                    cs = ct * CT
                    nc.scalar.copy(h2x[0:64, cs : cs + CT], z1[ct][0:64, :])
                    nc.scalar.copy(h2x[0:64, CB + cs : CB + cs + CT], z1[ct][64:128, :])
                    bo = blk * (CB // D) + ct * 32
                    nc.vector.reduce_sum(
                        out_sb[:, bo : bo + 32],
                        z1[ct][:].rearrange("p (b d) -> p b d", d=D),
                        axis=mybir.AxisListType.X,
                    )
                nc.gpsimd.dma_start(h2x[64:128, :], h2x[0:64, :])

                # per-block xdiag slice (single trigger, 1.05 MB)
                xd_sb = xdg.tile([128, NGB * 512], BF16, tag="xd", name=f"xd_{blk}")
                nc.sync.dma_start(xd_sb[:], xdiag_d[blk])
                xd_tiles[blk] = xd_sb

                # ---------------- layer 2 over this block ----------------
                z2 = [
                    zp.tile([128, CT], F32, tag="z", name=f"z2_{blk}_{ct}")
                    for ct in range(NCT)
                ]
                if blk + 1 < NBLK:
                    z1_cur = alloc_z1(blk + 1)
                    for u in range(NP1D):
                        emit_l1_dma(blk + 1, u)

                st_tiles = {}

                def emit_stream_dma(u):
                    xb2 = pairs.tile(
                        [128, 2 * CB], BF16, tag="xb", name=f"xb_{blk}_{u}"
                    )
                    q = nc.scalar if u % 2 == 0 else nc.sync
                    q.dma_start(xb2[:], xtp_d[blk, u])
                    st_tiles[u] = xb2

                for u in range(min(3, NSD)):
                    emit_stream_dma(u)
                for t in range(NPAIR):
                    if blk + 1 < NBLK and t < L1_CHUNKS:
                        emit_l1_mm(blk + 1, z1_cur, t)
                    u = t // 2
                    if u + 3 < NSD and t % 2 == 0:
                        emit_stream_dma(u + 3)
                    xb = st_tiles[u][:, (t % 2) * CB : (t % 2) * CB + CB]
                    # one fused TT per pair tile: multiplies both j-halves'
                    # duplicated H1 against the same xb (read twice via a
                    # stride-0 outer free dim).
                    p_sb = pkr.tile(
                        [128, 2 * CB], BF16, tag="p", name=f"p2_{blk}_{t}"
                    )
                    xb_rep = xb.unsqueeze(1).broadcast_to((128, 2, CB))
                    nc.vector.tensor_mul(
                        p_sb[:].rearrange("p (h c) -> p h c", h=2),
                        h2x[:].rearrange("p (h c) -> p h c", h=2),
                        xb_rep,
                    )
                    for half in range(2):
                        k = 2 * t + half
                        for ct in range(NCT):
                            nc.tensor.matmul(
                                z2[ct][:],
                                w1_sb[:, k * FN : (k + 1) * FN],
                                p_sb[
                                    :,
                                    half * CB + ct * CT : half * CB + (ct + 1) * CT,
                                ],
                                start=(k == 0),
                                stop=(k == L2_CHUNKS - 1),
                            )

                if blk == 0:
                    # stage the layer-3 consts behind block 0's stream DMAs
                    nc.scalar.dma_start(ident_sb[:], ident_d[:])
                    nc.scalar.dma_start(w2_sb[:], w2_d[:])

                for ct in range(NCT):
                    cc = c0 + ct * CT
                    nc.scalar.copy(h2_sb[:, cc : cc + CT], z2[ct][:])
                    bo = blk * (CB // D) + ct * 32
                    nc.vector.reduce_sum(
                        out_sb[:, BL + bo : BL + bo + 32],
                        z2[ct][:].rearrange("p (b d) -> p b d", d=D),
                        axis=mybir.AxisListType.X,
                    )
                # drain this block's layer-1/2 output columns early
                bo = blk * (CB // D)
                nc.sync.dma_start(
                    out_d[0][:, bo : bo + CB // D], out_sb[:, bo : bo + CB // D]
                )
                nc.sync.dma_start(
                    out_d[1][:, bo : bo + CB // D],
                    out_sb[:, BL + bo : BL + bo + CB // D],
                )

                # ------- layer 3, delayed one block for ScalarE ordering ---
                l3list = [blk - 1] if blk > 0 else []
                if blk == NBLK - 1:
                    l3list.append(blk)
                for l3blk in l3list:
                    hidx = l3blk // NBH
                    g2t_sb = g2t_tiles[hidx]
                    xd_cur = xd_tiles[l3blk]
                    # the final block's L3 runs in the kernel tail where
                    # VectorE is idle but ScalarE's in-order queue is the
                    # pacer — route its PSUM->SBUF copies to VectorE.
                    tail = l3blk == NBLK - 1

                    def l3copy(dst, src):
                        if tail:
                            nc.vector.tensor_scalar_add(dst, src, 0.0)
                        else:
                            nc.scalar.copy(dst, src)

                    for gl in range(NGB):
                        g = l3blk * NGB + gl
                        gh = (l3blk % NBH) * NGB + gl
                        ht_ps = l3ps.tile(
                            [128, 128], BF16, tag="l3", name=f"htps_{g}"
                        )
                        nc.tensor.transpose(
                            ht_ps[:], h2_sb[:, g * 128 : (g + 1) * 128], ident_sb[:]
                        )
                        ht_sb = hts.tile([128, 128], BF16, tag="hts", name=f"htsb_{g}")
                        l3copy(ht_sb[:], ht_ps[:])

                        g2_ps = l3ps.tile(
                            [128, 512], F32, tag="l3", name=f"g2ps_{g}"
                        )
                        nc.tensor.matmul(
                            g2_ps[:], ht_sb[:], xd_cur[:, gl * 512 : (gl + 1) * 512]
                        )
                        l3copy(
                            g2t_sb[:, gh * 512 : (gh + 1) * 512], g2_ps[:]
                        )

                    g2t_r = g2t_sb[:].rearrange(
                        "p (g b i) -> p g b i", b=8, i=F0
                    )
                    bi = l3blk % NBH
                    if hidx == 1:
                        if bi == 0:
                            o3_last = o3p.tile(
                                [128, 128], F32, tag="o3", name="o3_1"
                            )
                        for i in range(F0):
                            nc.tensor.matmul(
                                o3_last[:, bi * 64 : (bi + 1) * 64],
                                w2_sb[:, i * FN : (i + 1) * FN],
                                g2t_r[:, bi * NGB : (bi + 1) * NGB, :, i],
                                start=(i == 0),
                                stop=(i == F0 - 1),
                            )
                        o3_ps = o3_last
                    elif bi == NBH - 1:
                        o3_ps = o3p.tile(
                            [128, 128], F32, tag="o3", name=f"o3_{hidx}"
                        )
                        for i in range(F0):
                            nc.tensor.matmul(
                                o3_ps[:],
                                w2_sb[:, i * FN : (i + 1) * FN],
                                g2t_r[:, :, :, i],
                                start=(i == 0),
                                stop=(i == F0 - 1),
                            )
                    if l3blk % NBH == NBH - 1:
                        l3copy(
                            out_sb[:, 2 * BL + hidx * 128 : 2 * BL + (hidx + 1) * 128],
                            o3_ps[:],
                        )
                        nc.sync.dma_start(
                            out_d[2][:, hidx * 128 : (hidx + 1) * 128],
                            out_sb[:, 2 * BL + hidx * 128 : 2 * BL + (hidx + 1) * 128],
                        )

    nc.finalize()
    return nc


def _prep_inputs(x, W0, W1, W2):
    bf = ml_dtypes.bfloat16
    xs = np.ascontiguousarray(x).reshape(NCORES, BL, F0, D)

    def chunk_w(W, nchunk):
        Wc = W.reshape(nchunk, 128, FN).transpose(1, 0, 2).reshape(128, nchunk * FN)
        return np.ascontiguousarray(Wc).astype(bf)

    pi, pj = np.triu_indices(F0)                     # 2080 pairs, i <= j
    W0sym = np.zeros((L1_CHUNKS * 128, FN), dtype=np.float32)
    W0sym[:SYM_PAIRS] = W0[pi * F0 + pj]
    off = W0[pj * F0 + pi].copy()
    off[pi == pj] = 0.0
    W0sym[:SYM_PAIRS] += off
    w0c = chunk_w(W0sym, L1_CHUNKS)
    w2c = chunk_w(W2, F0)
    W1r = W1.reshape(F0, 2, 64, FN)          # [i, half, j_in_half, n]
    w1c = np.zeros((128, L2_CHUNKS * FN), dtype=bf)
    for t in range(NPAIR):
        for half in range(2):
            k = 2 * t + half
            w1c[0:64, k * FN : (k + 1) * FN] = W1r[2 * t, half].astype(bf)
            w1c[64:128, k * FN : (k + 1) * FN] = W1r[2 * t + 1, half].astype(bf)
    ident = np.eye(128, dtype=np.float32).astype(bf)

    i_idx = np.zeros(L1_CHUNKS * 128, dtype=np.int64)
    j_idx = np.zeros(L1_CHUNKS * 128, dtype=np.int64)
    i_idx[:SYM_PAIRS] = pi
    j_idx[:SYM_PAIRS] = pj

    in_maps = []
    for c in range(NCORES):
        xc = xs[c]                                   # [BL, F0, D]
        xt = xc.transpose(1, 0, 2).reshape(F0, C)    # [i, (b d)]
        xt_bf = xt.astype(bf)
        xt32 = xt_bf.astype(np.float32)

        p1 = (xt32[i_idx] * xt32[j_idx]).astype(bf)  # [2176, C]
        p1r = p1.reshape(L1_CHUNKS, 128, NBLK, CB)   # [t, p, blk, cb]
        xp1 = np.zeros((NBLK, NP1D, 128, 2 * CB), dtype=bf)
        for t in range(L1_CHUNKS):
            xp1[:, t // 2, :, (t % 2) * CB : (t % 2) * CB + CB] = (
                p1r[t].transpose(1, 0, 2)
            )

        xtb = xt_bf.reshape(F0, NBLK, CB)            # [i, blk, cb]
        xtp = np.zeros((NBLK, NSD, 128, 2 * CB), dtype=bf)
        for t in range(NPAIR):
            u, v = t // 2, t % 2
            rep = np.repeat(xtb[2 * t : 2 * t + 2], 64, axis=0)  # [128, blk, cb]
            xtp[:, u, :, v * CB : v * CB + CB] = rep.transpose(1, 0, 2)

        xd = np.zeros((8, D, NG, 8, F0), dtype=bf)
        xg = xc.reshape(NG, 8, F0, D)                # [g, bl, i, d]
        for bl in range(8):
            xd[bl, :, :, bl, :] = xg[:, bl].transpose(2, 0, 1).astype(bf)
        xdiag = (
            xd.reshape(128, NBLK, NGB * 512).transpose(1, 0, 2).copy()
        )

        in_maps.append(
            {
                "xp1": np.ascontiguousarray(xp1),
                "xtp": np.ascontiguousarray(xtp),
                "xdiag": np.ascontiguousarray(xdiag),
                "w0c": w0c,
                "w1c": np.ascontiguousarray(w1c),
                "w2c": w2c,
                "ident": ident,
            }
        )
    return in_maps


def _postprocess(results):
    outs = [
        np.asarray(r["out_nb"]).transpose(2, 0, 1).reshape(BL, 3 * FN)
        for r in results
    ]
    return np.ascontiguousarray(np.concatenate(outs, axis=0)).astype(np.float32)


def kernel(x, W0, W1, W2, _trace=False, _trace_kwargs=None):
    if "nc" not in _CACHE:
        _CACHE["nc"] = _build_program()
    nc = _CACHE["nc"]
    in_maps = _prep_inputs(
        np.asarray(x, dtype=np.float32),
        np.asarray(W0, dtype=np.float32),
        np.asarray(W1, dtype=np.float32),
        np.asarray(W2, dtype=np.float32),
    )
    kw = {}
    if _trace:
        kw["trace"] = True
        kw.update(_trace_kwargs or {})
    res = run_bass_kernel_spmd(nc, in_maps, core_ids=list(range(NCORES)), **kw)
    out = _postprocess(res.results)
    if _trace:
        _CACHE["last_results"] = res
    return out


# revision 16
# speedup vs baseline: 1.0770x; 1.0770x over previous
"""CIN (Compressed Interaction Network) Trainium2 kernel.

Baseline architecture (pair-chunk L2, host-streamed replicated tiles)
with DMA-efficiency and scheduling improvements:
  * 2-granule pre-interleaved host layouts: each stream DMA trigger
    moves 512KB ([128, 2*CB] contiguous per partition row) — halves
    the ~610ns-per-trigger issue cost on the DGE queues.
  * w1 staged via the GpSimd SWDGE queue at t=0 (needed by block 0's
    L2); w2/ident deferred behind block 0's stream DMAs (first needed
    by the l3blk=0 pass during block 1).
  * per-block single-trigger xdiag slices.
See the baseline kernel docstring for the full algorithm description.
"""

import sys

import numpy as np

try:
    import concourse.bass as bass  # noqa: F401
except ImportError:  # grading env fallback
    sys.path.insert(0, "/opt/trn_rl_repo")

import ml_dtypes
import concourse.bacc as bacc
import concourse.bass as bass
import concourse.mybir as mybir
import concourse.tile as tile
from concourse.bass_utils import run_bass_kernel_spmd

BF16 = mybir.dt.bfloat16
F32 = mybir.dt.float32

B, F0, D = 2048, 64, 16
NCORES = 8
BL = B // NCORES          # 256 batch rows per core
C = BL * D                # 4096 columns (b, d)
FN = 128                  # layer width (all three CIN layers)
CT = 512                  # matmul N tile (one PSUM bank of fp32)
CB = 1024                 # column block
NBLK = C // CB            # 4
NCT = CB // CT            # 2 column tiles per block
NPAIR = F0 // 2           # 32 pair tiles / L2 pair groups
NG = BL // 8              # 32 groups of 8 batch rows (layer-3 path)
NGB = CB // 128           # 8 layer-3 groups per block
NBH = NBLK // 2           # 2 blocks per layer-3 half
SYM_PAIRS = F0 * (F0 + 1) // 2          # 2080 unordered (i,j) pairs
L1_CHUNKS = (SYM_PAIRS + 127) // 128    # 17 (last chunk zero-padded)
L2_CHUNKS = F0                 # 64 (pair x j-half)

NSD = NPAIR // 2               # stream DMAs per block (2 pairs each): 16
NP1D = (L1_CHUNKS + 1) // 2    # xp1 DMAs per block (2 chunks each): 9

_CACHE = {}


def _build_program():
    nc = bacc.Bacc(None, target_bir_lowering=False)

    xp1_d = nc.dram_tensor("xp1", [NBLK, NP1D, 128, 2 * CB], BF16, kind="ExternalInput")
    xtp_d = nc.dram_tensor("xtp", [NBLK, NSD, 128, 2 * CB], BF16, kind="ExternalInput")
    xdiag_d = nc.dram_tensor("xdiag", [NBLK, 128, NGB * 512], BF16, kind="ExternalInput")
    w0_d = nc.dram_tensor("w0c", [128, L1_CHUNKS * FN], BF16, kind="ExternalInput")
    w1_d = nc.dram_tensor("w1c", [128, L2_CHUNKS * FN], BF16, kind="ExternalInput")
    w2_d = nc.dram_tensor("w2c", [128, F0 * FN], BF16, kind="ExternalInput")
    ident_d = nc.dram_tensor("ident", [128, 128], BF16, kind="ExternalInput")
    out_d = nc.dram_tensor("out_nb", [3, 128, BL], F32, kind="ExternalOutput")

    with tile.TileContext(nc) as tc:
        with (
            tc.tile_pool(name="const", bufs=1) as const,
            tc.tile_pool(name="hbuf", bufs=1) as hbuf,
            tc.tile_pool(name="outs", bufs=1) as outs,
            tc.tile_pool(name="p1s", bufs=7) as p1s,
            tc.tile_pool(name="pairs", bufs=7) as pairs,
            tc.tile_pool(name="h2x", bufs=2) as h2xp,
            tc.tile_pool(name="pkr", bufs=6) as pkr,
            tc.tile_pool(name="zp", bufs=5, space="PSUM") as zp,
            tc.tile_pool(name="l3sb", bufs=1) as l3sb,
            tc.tile_pool(name="l3ps", bufs=2, space="PSUM") as l3ps,
            tc.tile_pool(name="o3p", bufs=1, space="PSUM") as o3p,
            tc.tile_pool(name="hts", bufs=6) as hts,
            tc.tile_pool(name="xdg", bufs=2) as xdg,
        ):
            w0_sb = const.tile([128, L1_CHUNKS * FN], BF16)
            nc.scalar.dma_start(w0_sb[:], w0_d[:])
            w1_sb = const.tile([128, L2_CHUNKS * FN], BF16)
            nc.gpsimd.dma_start(w1_sb[:], w1_d[:])
            w2_sb = const.tile([128, F0 * FN], BF16)
            ident_sb = const.tile([128, 128], BF16)

            h2_sb = hbuf.tile([128, C], BF16, tag="h2")
            out_sb = outs.tile([128, 3 * BL], F32)

            # dense junk-matmul burst at kernel start: pulls the PE HAM
            # clock gate to 8/8 before the real accumulation chains begin.
            warm_sb = const.tile([128, 512], BF16)
            nc.vector.memset(warm_sb[:], 0.0)
            warm_ps = zp.tile([128, CT], F32, tag="z", name="warm_ps")
            for w in range(20):
                nc.tensor.matmul(
                    warm_ps[:],
                    warm_sb[:, 0:128],
                    warm_sb[:],
                    start=(w == 0),
                    stop=(w == 19),
                )

            def alloc_z1(blk):
                return [
                    zp.tile([128, CT], F32, tag="z", name=f"z1_{blk}_{ct}")
                    for ct in range(NCT)
                ]

            p1_tiles = {}

            # rows in the last (zero-padded) symmetrized L1 chunk
            L1_TAIL_ROWS = SYM_PAIRS - (L1_CHUNKS - 1) * 128   # 32

            def emit_l1_dma(blk, u):
                last = u == NP1D - 1 and L1_CHUNKS % 2 == 1
                if last:
                    # final chunk has only 32 live rows — skip the pad
                    p1 = p1s.tile([L1_TAIL_ROWS, CB], BF16, tag="p1",
                                  name=f"p1_{blk}_{u}")
                    q = nc.sync if u % 2 == 0 else nc.scalar
                    q.dma_start(p1[:], xp1_d[blk, u][0:L1_TAIL_ROWS, 0:CB])
                else:
                    p1 = p1s.tile([128, 2 * CB], BF16, tag="p1",
                                  name=f"p1_{blk}_{u}")
                    q = nc.sync if u % 2 == 0 else nc.scalar
                    q.dma_start(p1[:], xp1_d[blk, u])
                p1_tiles[(blk, u)] = p1

            def emit_l1_mm(blk, z1, t):
                p1 = p1_tiles[(blk, t // 2)]
                last = t == L1_CHUNKS - 1
                rows = L1_TAIL_ROWS if last else 128
                sl = p1[:, 0:CB] if last else p1[:, (t % 2) * CB : (t % 2) * CB + CB]
                for ct in range(NCT):
                    nc.tensor.matmul(
                        z1[ct][:],
                        w0_sb[0:rows, t * FN : (t + 1) * FN],
                        sl[0:rows, ct * CT : (ct + 1) * CT],
                        start=(t == 0),
                        stop=last,
                    )

            g2t_tiles = {}
            z1_cur = alloc_z1(0)
            for u in range(NP1D):
                emit_l1_dma(0, u)
            for t in range(L1_CHUNKS):
                emit_l1_mm(0, z1_cur, t)

            xd_tiles = {}
            for blk in range(NBLK):
                c0 = blk * CB
                half_idx = blk // NBH         # layer-3 half (0 or 1)
                if blk % NBH == 0:
                    g2t_tiles[half_idx] = l3sb.tile(
                        [128, NBH * NGB * 512],
                        BF16,
                        tag="g2t",
                        name=f"g2t_{half_idx}",
                    )
                z1 = z1_cur

                # z1 copy-out writes straight into the H2x duplication tile
                # (columns 0:CB hold the j<64 half, CB:2CB the j>=64 half);
                # one SBUF->SBUF DMA then fills partitions 64:128.
                h2x = h2xp.tile([128, 2 * CB], BF16, tag="h2x", name=f"h2x_{blk}")
                for ct in range(NCT):
                    cs = ct * CT
                    nc.scalar.copy(h2x[0:64, cs : cs + CT], z1[ct][0:64, :])
                    nc.scalar.copy(h2x[0:64, CB + cs : CB + cs + CT], z1[ct][64:128, :])
                    bo = blk * (CB // D) + ct * 32
                    nc.vector.reduce_sum(
                        out_sb[:, bo : bo + 32],
                        z1[ct][:].rearrange("p (b d) -> p b d", d=D),
                        axis=mybir.AxisListType.X,
                    )
                nc.gpsimd.dma_start(h2x[64:128, :], h2x[0:64, :])

                # ---------------- layer 2 over this block ----------------
                z2 = [
                    zp.tile([128, CT], F32, tag="z", name=f"z2_{blk}_{ct}")
                    for ct in range(NCT)
                ]
                if blk + 1 < NBLK:
                    z1_cur = alloc_z1(blk + 1)
                    for u in range(NP1D):
                        emit_l1_dma(blk + 1, u)

                st_tiles = {}

                def emit_stream_dma(u):
                    xb2 = pairs.tile(
                        [128, 2 * CB], BF16, tag="xb", name=f"xb_{blk}_{u}"
                    )
                    q = nc.scalar if u % 2 == 0 else nc.sync
                    q.dma_start(xb2[:], xtp_d[blk, u])
                    st_tiles[u] = xb2

                for u in range(min(3, NSD)):
                    emit_stream_dma(u)
                for t in range(NPAIR):
                    if blk + 1 < NBLK and t < L1_CHUNKS:
                        emit_l1_mm(blk + 1, z1_cur, t)
                    u = t // 2
                    if u + 3 < NSD and t % 2 == 0:
                        emit_stream_dma(u + 3)
                    xb = st_tiles[u][:, (t % 2) * CB : (t % 2) * CB + CB]
                    # one fused TT per pair tile: multiplies both j-halves'
                    # duplicated H1 against the same xb (read twice via a
                    # stride-0 outer free dim).
                    p_sb = pkr.tile(
                        [128, 2 * CB], BF16, tag="p", name=f"p2_{blk}_{t}"
                    )
                    xb_rep = xb.unsqueeze(1).broadcast_to((128, 2, CB))
                    nc.vector.tensor_mul(
                        p_sb[:].rearrange("p (h c) -> p h c", h=2),
                        h2x[:].rearrange("p (h c) -> p h c", h=2),
                        xb_rep,
                    )
                    for half in range(2):
                        k = 2 * t + half
                        for ct in range(NCT):
                            nc.tensor.matmul(
                                z2[ct][:],
                                w1_sb[:, k * FN : (k + 1) * FN],
                                p_sb[
                                    :,
                                    half * CB + ct * CT : half * CB + (ct + 1) * CT,
                                ],
                                start=(k == 0),
                                stop=(k == L2_CHUNKS - 1),
                            )

                if blk == 0:
                    # stage the layer-3 consts behind block 0's stream DMAs
                    nc.scalar.dma_start(ident_sb[:], ident_d[:])
                    nc.scalar.dma_start(w2_sb[:], w2_d[:])

                # per-block xdiag slice, emitted behind this block's streams
                # (first consumed by the delayed-by-one layer-3 pass)
                xd_sb = xdg.tile([128, NGB * 512], BF16, tag="xd", name=f"xd_{blk}")
                nc.sync.dma_start(xd_sb[:], xdiag_d[blk])
                xd_tiles[blk] = xd_sb

                for ct in range(NCT):
                    cc = c0 + ct * CT
                    nc.scalar.copy(h2_sb[:, cc : cc + CT], z2[ct][:])
                    bo = blk * (CB // D) + ct * 32
                    nc.vector.reduce_sum(
                        out_sb[:, BL + bo : BL + bo + 32],
                        z2[ct][:].rearrange("p (b d) -> p b d", d=D),
                        axis=mybir.AxisListType.X,
                    )
                # drain this block's layer-1/2 output columns early
                bo = blk * (CB // D)
                nc.sync.dma_start(
                    out_d[0][:, bo : bo + CB // D], out_sb[:, bo : bo + CB // D]
                )
                nc.sync.dma_start(
                    out_d[1][:, bo : bo + CB // D],
                    out_sb[:, BL + bo : BL + bo + CB // D],
                )

                # ------- layer 3, delayed one block for ScalarE ordering ---
                l3list = [blk - 1] if blk > 0 else []
                if blk == NBLK - 1:
                    l3list.append(blk)
                for l3blk in l3list:
                    hidx = l3blk // NBH
                    g2t_sb = g2t_tiles[hidx]
                    xd_cur = xd_tiles[l3blk]
                    # In the kernel tail (final block's pass) ScalarE's
                    # serial copy queue paces the PE: route the small ht
                    # copies to the idle VectorE so the two copy streams
                    # (ht on DVE, g2 on ScalarE) run in parallel.
                    tail = l3blk == NBLK - 1
                    for gl in range(NGB):
                        g = l3blk * NGB + gl
                        gh = (l3blk % NBH) * NGB + gl
                        ht_ps = l3ps.tile(
                            [128, 128], BF16, tag="l3", name=f"htps_{g}"
                        )
                        nc.tensor.transpose(
                            ht_ps[:], h2_sb[:, g * 128 : (g + 1) * 128], ident_sb[:]
                        )
                        ht_sb = hts.tile([128, 128], BF16, tag="hts", name=f"htsb_{g}")
                        if tail:
                            nc.vector.tensor_scalar_add(ht_sb[:], ht_ps[:], 0.0)
                        else:
                            nc.scalar.copy(ht_sb[:], ht_ps[:])

                        g2_ps = l3ps.tile(
                            [128, 512], F32, tag="l3", name=f"g2ps_{g}"
                        )
                        nc.tensor.matmul(
                            g2_ps[:], ht_sb[:], xd_cur[:, gl * 512 : (gl + 1) * 512]
                        )
                        nc.scalar.copy(
                            g2t_sb[:, gh * 512 : (gh + 1) * 512], g2_ps[:]
                        )

                    g2t_r = g2t_sb[:].rearrange(
                        "p (g b i) -> p g b i", b=8, i=F0
                    )
                    bi = l3blk % NBH
                    if hidx == 1:
                        if bi == 0:
                            o3_last = o3p.tile(
                                [128, 128], F32, tag="o3", name="o3_1"
                            )
                        for i in range(F0):
                            nc.tensor.matmul(
                                o3_last[:, bi * 64 : (bi + 1) * 64],
                                w2_sb[:, i * FN : (i + 1) * FN],
                                g2t_r[:, bi * NGB : (bi + 1) * NGB, :, i],
                                start=(i == 0),
                                stop=(i == F0 - 1),
                            )
                        o3_ps = o3_last
                    elif bi == NBH - 1:
                        o3_ps = o3p.tile(
                            [128, 128], F32, tag="o3", name=f"o3_{hidx}"
                        )
                        for i in range(F0):
                            nc.tensor.matmul(
                                o3_ps[:],
                                w2_sb[:, i * FN : (i + 1) * FN],
                                g2t_r[:, :, :, i],
                                start=(i == 0),
                                stop=(i == F0 - 1),
                            )
                    if l3blk % NBH == NBH - 1:
                        nc.scalar.copy(
                            out_sb[:, 2 * BL + hidx * 128 : 2 * BL + (hidx + 1) * 128],
                            o3_ps[:],
                        )
                        nc.sync.dma_start(
                            out_d[2][:, hidx * 128 : (hidx + 1) * 128],
                            out_sb[:, 2 * BL + hidx * 128 : 2 * BL + (hidx + 1) * 128],
                        )

    nc.finalize()
    return nc


def _prep_inputs(x, W0, W1, W2):
    bf = ml_dtypes.bfloat16
    xs = np.ascontiguousarray(x).reshape(NCORES, BL, F0, D)

    def chunk_w(W, nchunk):
        Wc = W.reshape(nchunk, 128, FN).transpose(1, 0, 2).reshape(128, nchunk * FN)
        return np.ascontiguousarray(Wc).astype(bf)

    pi, pj = np.triu_indices(F0)                     # 2080 pairs, i <= j
    W0sym = np.zeros((L1_CHUNKS * 128, FN), dtype=np.float32)
    W0sym[:SYM_PAIRS] = W0[pi * F0 + pj]
    off = W0[pj * F0 + pi].copy()
    off[pi == pj] = 0.0
    W0sym[:SYM_PAIRS] += off
    w0c = chunk_w(W0sym, L1_CHUNKS)
    w2c = chunk_w(W2, F0)
    W1r = W1.reshape(F0, 2, 64, FN)          # [i, half, j_in_half, n]
    w1c = np.zeros((128, L2_CHUNKS * FN), dtype=bf)
    for t in range(NPAIR):
        for half in range(2):
            k = 2 * t + half
            w1c[0:64, k * FN : (k + 1) * FN] = W1r[2 * t, half].astype(bf)
            w1c[64:128, k * FN : (k + 1) * FN] = W1r[2 * t + 1, half].astype(bf)
    ident = np.eye(128, dtype=np.float32).astype(bf)

    i_idx = np.zeros(L1_CHUNKS * 128, dtype=np.int64)
    j_idx = np.zeros(L1_CHUNKS * 128, dtype=np.int64)
    i_idx[:SYM_PAIRS] = pi
    j_idx[:SYM_PAIRS] = pj

    in_maps = []
    for c in range(NCORES):
        xc = xs[c]                                   # [BL, F0, D]
        xt = xc.transpose(1, 0, 2).reshape(F0, C)    # [i, (b d)]
        xt_bf = xt.astype(bf)
        xt32 = xt_bf.astype(np.float32)

        p1 = (xt32[i_idx] * xt32[j_idx]).astype(bf)  # [2176, C]
        p1r = p1.reshape(L1_CHUNKS, 128, NBLK, CB)   # [t, p, blk, cb]
        xp1 = np.zeros((NBLK, NP1D, 128, 2 * CB), dtype=bf)
        for t in range(L1_CHUNKS):
            xp1[:, t // 2, :, (t % 2) * CB : (t % 2) * CB + CB] = (
                p1r[t].transpose(1, 0, 2)
            )

        xtb = xt_bf.reshape(F0, NBLK, CB)            # [i, blk, cb]
        xtp = np.zeros((NBLK, NSD, 128, 2 * CB), dtype=bf)
        for t in range(NPAIR):
            u, v = t // 2, t % 2
            rep = np.repeat(xtb[2 * t : 2 * t + 2], 64, axis=0)  # [128, blk, cb]
            xtp[:, u, :, v * CB : v * CB + CB] = rep.transpose(1, 0, 2)

        xd = np.zeros((8, D, NG, 8, F0), dtype=bf)
        xg = xc.reshape(NG, 8, F0, D)                # [g, bl, i, d]
        for bl in range(8):
            xd[bl, :, :, bl, :] = xg[:, bl].transpose(2, 0, 1).astype(bf)
        xdiag = (
            xd.reshape(128, NBLK, NGB * 512).transpose(1, 0, 2).copy()
        )

        in_maps.append(
            {
                "xp1": np.ascontiguousarray(xp1),
                "xtp": np.ascontiguousarray(xtp),
                "xdiag": np.ascontiguousarray(xdiag),
                "w0c": w0c,
                "w1c": np.ascontiguousarray(w1c),
                "w2c": w2c,
                "ident": ident,
            }
        )
    return in_maps


def _postprocess(results):
    outs = [
        np.asarray(r["out_nb"]).transpose(2, 0, 1).reshape(BL, 3 * FN)
        for r in results
    ]
    return np.ascontiguousarray(np.concatenate(outs, axis=0)).astype(np.float32)


def kernel(x, W0, W1, W2, _trace=False, _trace_kwargs=None):
    if "nc" not in _CACHE:
        _CACHE["nc"] = _build_program()
    nc = _CACHE["nc"]
    in_maps = _prep_inputs(
        np.asarray(x, dtype=np.float32),
        np.asarray(W0, dtype=np.float32),
        np.asarray(W1, dtype=np.float32),
        np.asarray(W2, dtype=np.float32),
    )
    kw = {}
    if _trace:
        kw["trace"] = True
        kw.update(_trace_kwargs or {})
    res = run_bass_kernel_spmd(nc, in_maps, core_ids=list(range(NCORES)), **kw)
    out = _postprocess(res.results)
    if _trace:
        _CACHE["last_results"] = res
    return out
